# revision 38
# baseline (speedup 1.0000x reference)
"""DeltaNet Trainium2 kernel — 8-core SPMD, one (batch, head) pair per core.

v3: single-scope pools so phase B (projections+conv) overlaps phase D (chunked
delta rule) under the dataflow Tile scheduler; serial chain emitted at high
priority so it advances while projection bulk work fills PE gaps.  All
per-position scalar machinery is s-major ([128 positions, chunk] columns fed
by ap=1 matmuls) instead of [1,S] rows: beta via tanh (same act table set as
silu), l2-norm sums via ones-matvec on squared planes, 1/sqrt via DVE
Newton iterations from a bit-shift seed.  Exactly one activation-table set
(silu/tanh/square/copy) is used, eliminating table reloads.  RMS-norm scale
is folded into the transposed-o drain; o_proj PSUM drains are plain copies
split between Act and GpSimd; output DMA goes through HWDGE.
"""

import os
import sys
from contextlib import ExitStack

import ml_dtypes
import numpy as np

for _p in ("/opt/trn_rl_repo", "/root/.axon_site/_ro/trn_rl_repo"):
    if os.path.isdir(_p) and _p not in sys.path:
        sys.path.insert(0, _p)

import concourse.bass as bass  # noqa: E402
import concourse.tile as tile  # noqa: E402
from concourse import bacc, mybir  # noqa: E402
from concourse.bass_utils import run_bass_kernel_spmd  # noqa: E402

F32 = mybir.dt.float32
I32 = mybir.dt.int32
BF16 = mybir.dt.bfloat16
AF = mybir.ActivationFunctionType
OP = mybir.AluOpType

HID = 1024
D = 256
C = 128
KT = HID // 128
NH = 4
B = 2
S_FULL = 2048
LOOKAHEAD = 2
RSQRT_C = 0x5F3759DF

TAGS = {}


def _tag(inst, label):
    try:
        TAGS[inst.ins.name] = label
    except Exception:
        pass
    return inst


def build_nc(nchunk=S_FULL // C, dbg=False):
    S = nchunk * C
    scs = 512 if S >= 512 else S
    nsc = S // scs
    nc = bacc.Bacc("TRN2", target_bir_lowering=False, debug=False)

    xt_d = nc.dram_tensor("xt", [HID, S], BF16, kind="ExternalInput")
    wq_d = nc.dram_tensor("wq", [HID, D], BF16, kind="ExternalInput")
    wk_d = nc.dram_tensor("wk", [HID, D], BF16, kind="ExternalInput")
    wv_d = nc.dram_tensor("wv", [HID, D], BF16, kind="ExternalInput")
    wb_d = nc.dram_tensor("wb", [HID, 1], BF16, kind="ExternalInput")
    wo_d = nc.dram_tensor("wo", [D, HID], BF16, kind="ExternalInput")
    cdq_d = nc.dram_tensor("cdq", [128, 8 * 128], BF16, kind="ExternalInput")
    cdk_d = nc.dram_tensor("cdk", [128, 8 * 128], BF16, kind="ExternalInput")
    cdv_d = nc.dram_tensor("cdv", [128, 8 * 128], BF16, kind="ExternalInput")
    identb_d = nc.dram_tensor("identb", [128, 128], BF16, kind="ExternalInput")
    identb2_d = nc.dram_tensor("identb2", [128, 256], BF16, kind="ExternalInput")
    identb4_d = nc.dram_tensor("identb4", [128, 512], BF16, kind="ExternalInput")
    onescol_d = nc.dram_tensor("onescol", [128, 1], BF16, kind="ExternalInput")
    mlow_d = nc.dram_tensor("mlow", [128, 128], F32, kind="ExternalInput")
    mup_d = nc.dram_tensor("mup", [128, 128], F32, kind="ExternalInput")
    out_d = nc.dram_tensor("out", [S, HID], F32, kind="ExternalOutput")
    dbgsc_d = None
    if dbg:
        dbgsc_d = nc.dram_tensor("dbgsc", [128, 6 * nchunk], F32,
                                 kind="ExternalOutput")
        dbgo_d = nc.dram_tensor("dbgo", [128, 256 * nchunk], BF16,
                                kind="ExternalOutput")

    with tile.TileContext(nc) as tc, ExitStack() as ctx:
        # ---------------- pools (single scope: phases overlap) ----------
        # PSUM budget (8 banks): ppb 2 + ppc 1 + ppt 2 + ppS 1 + ppw 1 +
        # ppo 1 = 8.
        pconst = ctx.enter_context(tc.tile_pool(name="pconst", bufs=1))
        pplane = ctx.enter_context(tc.tile_pool(name="pplane", bufs=1))
        pw = ctx.enter_context(tc.tile_pool(name="pw", bufs=1))
        pxt = ctx.enter_context(tc.tile_pool(name="pxt", bufs=1))
        pwt = ctx.enter_context(tc.tile_pool(name="pwt", bufs=2))
        pdiag = ctx.enter_context(tc.tile_pool(name="pdiag", bufs=1))
        praw = ctx.enter_context(tc.tile_pool(name="praw", bufs=1))
        ppb = ctx.enter_context(tc.tile_pool(name="ppb", bufs=2, space="PSUM"))
        WIN = LOOKAHEAD + 2
        pS = ctx.enter_context(tc.tile_pool(name="pS", bufs=2))
        pcs = ctx.enter_context(tc.tile_pool(name="pcs", bufs=2))
        pcm = ctx.enter_context(tc.tile_pool(name="pcm", bufs=2))
        pwin = ctx.enter_context(tc.tile_pool(name="pwin", bufs=WIN))
        pout = ctx.enter_context(tc.tile_pool(name="pout", bufs=2))
        ppS = ctx.enter_context(tc.tile_pool(name="ppS", bufs=1, space="PSUM"))
        ppwo = ctx.enter_context(tc.tile_pool(name="ppwo", bufs=1, space="PSUM"))
        ppt = ctx.enter_context(tc.tile_pool(name="ppt", bufs=2, space="PSUM"))
        ppop = ctx.enter_context(tc.tile_pool(name="ppop", bufs=1, space="PSUM"))
        pcsq = ctx.enter_context(tc.tile_pool(name="pcsq", bufs=3))
        pcrow = ctx.enter_context(tc.tile_pool(name="pcrow", bufs=2))

        identb = pconst.tile([128, 128], BF16)
        identb2 = pconst.tile([128, 256], BF16)
        identb4 = pconst.tile([128, 512], BF16)
        onescol = pconst.tile([128, 1], BF16)
        mlow = pconst.tile([128, 128], F32)
        mup = pconst.tile([128, 128], F32)

        # s-major per-position scalar columns, one column per chunk
        beta_c = pconst.tile([128, nchunk], F32)
        ak_c = pconst.tile([128, nchunk], F32)
        bk_c = pconst.tile([128, nchunk], F32)
        nbk2_c = pconst.tile([128, nchunk], F32)
        e5q_c = pconst.tile([128, nchunk], F32)
        m_c = pconst.tile([128, nchunk], F32)
        rs_c = pconst.tile([128, nchunk], F32)
        sums_c = pconst.tile([128, nchunk], F32)

        wo_sb = pw.tile([128, 2, HID], BF16)
        wb_sb = pw.tile([128, KT, 1], BF16)

        # planes: kq{dt} holds k in [:,0,:] and q in [:,1,:]; v separate
        kq0 = pplane.tile([128, 2, S], BF16, name="kq0")
        kq1 = pplane.tile([128, 2, S], BF16, name="kq1")
        v0 = pplane.tile([128, S], BF16, name="v0")
        v1 = pplane.tile([128, S], BF16, name="v1")

        xt_sb = pxt.tile([128, KT, S], BF16)

        def rsqrt_emit(dst, x, rounds, t1, t2):
            """dst = 1/sqrt(x) via shift seed + `rounds` Newton steps.

            x, dst, t1, t2: f32 APs of identical shape; t1/t2 scratch."""
            nc.vector.tensor_scalar(
                out=t1.bitcast(I32), in0=x.bitcast(I32), scalar1=1,
                scalar2=None, op0=OP.logical_shift_right,
            )
            nc.vector.tensor_scalar(
                out=t1.bitcast(I32), in0=t1.bitcast(I32), scalar1=-1,
                scalar2=RSQRT_C, op0=OP.mult, op1=OP.add,
            )
            y = t1
            for r in range(rounds):
                nc.vector.tensor_mul(t2, y, y)
                nc.vector.tensor_mul(t2, t2, x)
                nc.vector.tensor_scalar(
                    out=t2, in0=t2, scalar1=-0.5, scalar2=1.5,
                    op0=OP.mult, op1=OP.add,
                )
                nc.vector.tensor_mul(dst if r == rounds - 1 else y, y, t2)

        # ---------------- phase B: projections + conv + silu -------------
        # Emitted per 512-column slab (emit_slab_B below) so the dataflow
        # scheduler can overlap projection bulk work with the delta-rule
        # chain — emission order sets priorities.
        nc.sync.dma_start(
            out=wb_sb, in_=wb_d.ap().rearrange("(k p) o -> p k o", p=128)
        )
        wd_srcs = {"q": wq_d, "k": wk_d, "v": wv_d}
        w_sbs, diags, raws = {}, {}, {}
        for t in ("k", "q", "v"):
            w_sbs[t] = pwt.tile(
                [128, KT, D], BF16, tag="w", name=f"w_{t}", bufs=3
            )
            nc.sync.dma_start(
                out=w_sbs[t],
                in_=wd_srcs[t].ap().rearrange("(k p) d -> p k d", p=128),
            )
        for t in ("k", "q", "v"):
            cd_d = {"q": cdq_d, "k": cdk_d, "v": cdv_d}[t]
            diags[t] = pdiag.tile(
                [128, 8 * 128], BF16, tag=f"diag_{t}", name=f"diag_{t}"
            )
            nc.sync.dma_start(out=diags[t], in_=cd_d.ap())
        xt_src = xt_d.ap().rearrange("(k p) s -> p k s", p=128)
        # slab 0 per-kk so its compute starts after ~1/4 of the xt load;
        # the remainder batched per kk
        for kk in range(KT):
            nc.sync.dma_start(
                out=xt_sb[:, kk, 0:scs], in_=xt_src[:, kk, 0:scs]
            )
        for kk in range(KT):
            nc.sync.dma_start(
                out=xt_sb[:, kk, scs:S], in_=xt_src[:, kk, scs:S]
            )
        for t in ("k", "q", "v"):
            for dt_ in range(2):
                raws[(t, dt_)] = praw.tile(
                    [128, S + 8], BF16, tag=f"raw_{t}{dt_}", name=f"raw_{t}{dt_}"
                )
                nc.gpsimd.memset(raws[(t, dt_)][:, 0:8], 0.0)

        copy_flip = [0]

        def emit_slab_B(sc):
            base = sc * scs
            for t in ("k", "q", "v"):
                w_sb = w_sbs[t]
                diag = diags[t]
                for dt_ in range(2):
                    raw = raws[(t, dt_)]
                    ps = ppb.tile([128, scs], F32, tag="ps", name="psraw")
                    for kk in range(KT):
                        nc.tensor.matmul(
                            ps,
                            w_sb[:, kk, dt_ * 128 : (dt_ + 1) * 128],
                            xt_sb[:, kk, base : base + scs],
                            start=(kk == 0),
                            stop=(kk == KT - 1),
                        )
                    dst = raw[:, 8 + base : 8 + base + scs]
                    if copy_flip[0] % 2 == 0:
                        nc.vector.tensor_copy(dst, ps)
                    else:
                        nc.scalar.activation(out=dst, in_=ps, func=AF.Copy)
                    copy_flip[0] += 1
                    # conv (4 taps as diagonal-stationary matmuls) + SiLU
                    if t == "v":
                        sdst = (v0, v1)[dt_][:, base : base + scs]
                    else:
                        ti = 0 if t == "k" else 1
                        sdst = (kq0, kq1)[dt_][:, ti, base : base + scs]
                    psc = ppb.tile([128, scs], F32, tag="ps", name="psconv")
                    for j in (3, 2, 1, 0):
                        sh = 3 - j
                        dslc = diag[
                            :, (j * 2 + dt_) * 128 : (j * 2 + dt_ + 1) * 128
                        ]
                        nc.tensor.matmul(
                            psc,
                            dslc,
                            raw[:, 8 + base - sh : 8 + base + scs - sh],
                            start=(j == 3),
                            stop=(j == 0),
                        )
                    _tag(nc.scalar.activation(out=sdst, in_=psc,
                                              func=AF.Silu),
                         f"silu_{t}{dt_}_s{sc}")

        # deferred small DMAs
        nc.sync.dma_start(out=identb, in_=identb_d.ap())
        nc.sync.dma_start(out=identb2, in_=identb2_d.ap())
        nc.sync.dma_start(out=identb4, in_=identb4_d.ap())
        nc.sync.dma_start(out=onescol, in_=onescol_d.ap())
        nc.sync.dma_start(out=mlow, in_=mlow_d.ap())
        nc.sync.dma_start(out=mup, in_=mup_d.ap())
        nc.sync.dma_start(
            out=wo_sb, in_=wo_d.ap().rearrange("(t p) h -> p t h", p=128)
        )

        # ---------------- phase C: s-major scalar columns per slab --------
        def emit_scalars(q):
            sl4 = slice(4 * q, 4 * q + 4)
            slab = slice(q * scs, (q + 1) * scs)
            psr = ppop.tile([128, 16], F32, tag="rows", name="psr")
            # beta = sigmoid(x Wb): ap=1 matmuls, one column per chunk
            for cc in range(4):
                i = 4 * q + cc
                ch = slice(i * C, (i + 1) * C)
                for kk in range(KT):
                    nc.tensor.matmul(
                        psr[:, cc : cc + 1],
                        xt_sb[:, kk, ch],
                        wb_sb[:, kk, :],
                        start=(kk == 0),
                        stop=(kk == KT - 1),
                    )
            # squared k/q planes for the slab, then ones-matvec per chunk
            sqs = {}
            for ti, t in ((0, "k"), (1, "q")):
                for dt_ in range(2):
                    kqp = (kq0, kq1)[dt_]
                    sq = pcsq.tile(
                        [128, scs], BF16, tag="sq", name=f"sq_{t}{dt_}", bufs=4
                    )
                    _tag(nc.vector.tensor_mul(
                        sq, kqp[:, ti, slab], kqp[:, ti, slab]
                    ), f"sq_q{q}_{ti}{dt_}")
                    sqs[(ti, dt_)] = sq
            for ti in (0, 1):
                for cc in range(4):
                    col = 4 + 4 * ti + cc
                    ch128 = slice(cc * C, (cc + 1) * C)
                    for dt_ in range(2):
                        nc.tensor.matmul(
                            psr[:, col : col + 1],
                            sqs[(ti, dt_)][:, ch128],
                            onescol,
                            start=(dt_ == 0),
                            stop=(dt_ == 1),
                        )
            # drains: beta via tanh (same act set as silu), sums via DVE
            th = pcrow.tile([128, 4], F32, tag="th", name="th")
            _tag(nc.scalar.activation(out=th, in_=psr[:, 0:4], func=AF.Tanh,
                                 scale=0.5), f"tanh_q{q}")
            nc.vector.tensor_scalar(
                out=beta_c[:, sl4], in0=th, scalar1=0.5, scalar2=0.5,
                op0=OP.mult, op1=OP.add,
            )
            sk = pcrow.tile([128, 4], F32, tag="sk", name="sk")
            nc.vector.tensor_scalar(
                out=sk, in0=psr[:, 4:8], scalar1=1e-6, scalar2=None,
                op0=OP.add,
            )
            nc.vector.tensor_scalar(
                out=e5q_c[:, sl4], in0=psr[:, 8:12], scalar1=1e-6,
                scalar2=1e-5, op0=OP.add, op1=OP.mult,
            )
            t1 = pcrow.tile([128, 4], F32, tag="t1", name="t1")
            t2 = pcrow.tile([128, 4], F32, tag="t2", name="t2")
            rsqrt_emit(ak_c[:, sl4], sk[:, 0:4], 2, t1[:, 0:4], t2[:, 0:4])
            nc.vector.tensor_mul(bk_c[:, sl4], beta_c[:, sl4], ak_c[:, sl4])
            _tag(nc.vector.scalar_tensor_tensor(
                out=nbk2_c[:, sl4], in0=bk_c[:, sl4], scalar=-1.0,
                in1=ak_c[:, sl4], op0=OP.mult, op1=OP.mult,
            ), f"nbk2_q{q}")

        psS0 = ppS.tile([128, 256], F32, tag="psS0", name="psS0")
        psS1 = ppS.tile([128, 256], F32, tag="psS1", name="psS1")

        state = {}

        def emit_pass1_quad(qi):
            """Chunks 4qi..4qi+3 share [128,512] quad tiles through the
            Neumann ladder — one PSUM-drain copy advances four ladders."""
            cis = [4 * qi + cc for cc in range(4)]
            chs = [slice(i * C, (i + 1) * C) for i in cis]

            psAH = []
            for cc in range(4):
                p = ppt.tile([128, 256], F32, tag="ps", name=f"psAH{cc}")
                _tag(nc.tensor.matmul(p, kq0[:, 0, chs[cc]], kq0[:, :, chs[cc]],
                                 start=True, stop=False), f"psAH_q{qi}_{cc}a")
                _tag(nc.tensor.matmul(p, kq1[:, 0, chs[cc]], kq1[:, :, chs[cc]],
                                 start=False, stop=True), f"psAH_q{qi}_{cc}b")
                psAH.append(p)
            X = pcs.tile([128, 512], BF16, tag="X", name="X", bufs=2)
            Hm = pwin.tile([128, 512], BF16, tag="Hm", name="Hm", bufs=4)
            for cc in range(4):
                h = slice(cc * 128, (cc + 1) * 128)
                i = cis[cc]
                _tag(nc.vector.scalar_tensor_tensor(
                    out=X[:, h], in0=psAH[cc][:, 0:128],
                    scalar=nbk2_c[:, i : i + 1], in1=mlow,
                    op0=OP.mult, op1=OP.mult,
                ), f"Xstt_q{qi}_{cc}")
                nc.vector.tensor_mul(Hm[:, h], psAH[cc][:, 128:256], mup)

            def quadmm(lhsP, rhsP, name):
                ps = ppt.tile([128, 512], F32, tag="ps", name=name)
                for cc in range(4):
                    h = slice(cc * 128, (cc + 1) * 128)
                    nc.tensor.matmul(ps[:, h], lhsP[:, h], rhsP[:, h],
                                     start=True, stop=True)
                return ps

            def quadtrans(src, name):
                ps = ppt.tile([128, 512], BF16, tag="ps", name=name)
                for cc in range(4):
                    h = slice(cc * 128, (cc + 1) * 128)
                    nc.tensor.transpose(ps[:, h], src[:, h], identb)
                return ps

            psZ = quadtrans(X, "psZ")
            _tag(psZ._last_write if hasattr(psZ, '_last_write') else None, "na")
            Z = pcs.tile([128, 512], BF16, tag="Z", name="Z", bufs=2)
            nc.scalar.activation(out=Z, in_=psZ, func=AF.Copy)
            ZI = pcs.tile([128, 512], BF16, tag="ZI", name="ZI", bufs=2)
            nc.vector.tensor_add(ZI, psZ, identb4)

            psX2 = quadmm(Z, X, "psX2")
            X2 = pcs.tile([128, 512], BF16, tag="X2", name="X2", bufs=2)
            nc.scalar.activation(out=X2, in_=psX2, func=AF.Copy)
            X2I = pcs.tile([128, 512], BF16, tag="X2I", name="X2I", bufs=2)
            nc.vector.tensor_add(X2I, psX2, identb4)

            psZ2 = quadmm(X, Z, "psZ2")
            Z2 = pcs.tile([128, 512], BF16, tag="Z2", name="Z2", bufs=2)
            nc.scalar.activation(out=Z2, in_=psZ2, func=AF.Copy)

            psX4 = quadmm(Z2, X2, "psX4")
            X4I = pcs.tile([128, 512], BF16, tag="X4I", name="X4I", bufs=2)
            nc.vector.tensor_add(X4I, psX4, identb4)

            # (I+Z2)(I+Z) = (X2I)^T @ ZI directly in Z-space — skips the
            # XB transpose hop of the previous formulation
            psTZ2 = quadmm(X2I, ZI, "psTZ2")
            TZ2 = pcs.tile([128, 512], BF16, tag="XB", name="TZ2", bufs=2)
            nc.scalar.activation(out=TZ2, in_=psTZ2, func=AF.Copy)

            # Tt = T^T = (I+Z4)(I+Z2)(I+Z) = X4I^T @ TZ2 per chunk; lhsT roles
            # below give u = Tt^T vb = T vb and Gt = Kb^T Tt = (T Kb)^T.
            psT = quadmm(X4I, TZ2, "psT")
            T = pcs.tile([128, 512], BF16, tag="T", name="T", bufs=2)
            _tag(nc.scalar.activation(out=T, in_=psT, func=AF.Copy),
                 f"Tcopy_q{qi}")

            vbs, ktoks, Kbs = [], [], []
            for cc in range(4):
                ch = chs[cc]
                i = cis[cc]
                psVK = ppt.tile([128, 512], BF16, tag="ps", name=f"psVK{cc}")
                nc.tensor.transpose(psVK[:, 0:128], v0[:, ch], identb)
                nc.tensor.transpose(psVK[:, 128:256], v1[:, ch], identb)
                nc.tensor.transpose(psVK[:, 256:384], kq0[:, 0, ch], identb)
                nc.tensor.transpose(psVK[:, 384:512], kq1[:, 0, ch], identb)
                vb = pcm.tile([128, 256], BF16, tag="vb", name="vb", bufs=5)
                nc.vector.tensor_scalar(
                    out=vb, in0=psVK[:, 0:256],
                    scalar1=bk_c[:, i : i + 1], scalar2=None,
                    op0=OP.mult,
                )
                ktok = pwin.tile([128, 256], BF16, tag="ktok", name="ktok",
                                 bufs=16)
                nc.scalar.activation(out=ktok, in_=psVK[:, 256:512],
                                     func=AF.Copy)
                vbs.append(vb)
                ktoks.append(ktok)
                if i > 0:
                    Kb = pcm.tile([128, 256], BF16, tag="Kb", name="Kb", bufs=5)
                    nc.vector.tensor_scalar(
                        out=Kb, in0=psVK[:, 256:512],
                        scalar1=nbk2_c[:, i : i + 1], scalar2=None,
                        op0=OP.mult,
                    )
                    Kbs.append(Kb)
                else:
                    Kbs.append(None)

            us, Gts = [], []
            for half in range(2):
                psU = ppt.tile([128, 512], F32, tag="ps", name=f"psU{half}")
                for j in range(2):
                    cc = 2 * half + j
                    nc.tensor.matmul(
                        psU[:, j * 256 : (j + 1) * 256],
                        T[:, cc * 128 : (cc + 1) * 128], vbs[cc],
                        start=True, stop=True,
                    )
                u = pwin.tile([128, 512], BF16, tag="u", name="u", bufs=8)
                _tag(nc.scalar.activation(out=u, in_=psU, func=AF.Copy),
                     f"ucopy_q{qi}_{half}")
                us.append(u)

                psGt = ppt.tile([128, 512], F32, tag="ps", name=f"psGt{half}")
                wrote = False
                for j in range(2):
                    cc = 2 * half + j
                    if Kbs[cc] is None:
                        continue
                    wrote = True
                    for dh in range(2):
                        nc.tensor.matmul(
                            psGt[:, j * 256 + dh * 128 : j * 256 + (dh + 1) * 128],
                            Kbs[cc][:, dh * 128 : (dh + 1) * 128],
                            T[:, cc * 128 : (cc + 1) * 128],
                            start=True, stop=True,
                        )
                Gt = pwin.tile([128, 512], BF16, tag="Gt", name="Gt", bufs=8)
                if wrote:
                    if half == 0 and qi == 0:
                        nc.scalar.activation(out=Gt[:, 256:512],
                                             in_=psGt[:, 256:512], func=AF.Copy)
                    else:
                        nc.scalar.activation(out=Gt, in_=psGt, func=AF.Copy)
                Gts.append(Gt)

            for cc in range(4):
                i = cis[cc]
                half, j = cc // 2, cc % 2
                state[i] = dict(
                    Hm=Hm[:, cc * 128 : (cc + 1) * 128],
                    ktok=ktoks[cc],
                    u=us[half][:, j * 256 : (j + 1) * 256],
                    Gt0=Gts[half][:, j * 256 : j * 256 + 128],
                    Gt1=Gts[half][:, j * 256 + 128 : (j + 1) * 256],
                )

        def emit_chain(i):
            with tc.high_priority():
                _emit_chain(i)

        def _emit_chain(i):
            ch = slice(i * C, (i + 1) * C)
            st = state[i]
            wo_ps = ppwo.tile([128, 512], F32, tag="wo", name="wo_ps")
            S_sb = None
            if i > 0:
                S_sb = pS.tile([128, 512], BF16, tag="S", name="S_sb")
                _tag(nc.scalar.activation(out=S_sb[:, 0:256], in_=psS0,
                                          func=AF.Copy), f"Ssb_a_{i}")
                _tag(nc.vector.tensor_copy(S_sb[:, 256:512], psS1),
                     f"Ssb_d_{i}")

            if i > 0:
                psW = wo_ps[:, 0:256]
                nc.tensor.matmul(
                    psW, st["Gt0"], S_sb[:, 0:256],
                    start=True, stop=False,
                )
                nc.tensor.matmul(
                    psW, st["Gt1"], S_sb[:, 256:512],
                    start=False, stop=True,
                )
                w = pcm.tile([128, 256], BF16, tag="w", name="w", bufs=3)
                _tag(nc.vector.tensor_add(w, psW, st["u"]), f"wadd_{i}")
            else:
                w = st["u"]

            # state update first so the chain keeps moving
            _tag(nc.tensor.matmul(
                psS0, st["ktok"][:, 0:128], w,
                start=(i == 0), stop=(i == nchunk - 1), skip_group_check=True,
            ), f"psS0_{i}")
            _tag(nc.tensor.matmul(
                psS1, st["ktok"][:, 128:256], w,
                start=(i == 0), stop=(i == nchunk - 1), skip_group_check=True,
            ), f"psS1_{i}")

            pso = wo_ps[:, 256:512]
            if i > 0:
                nc.tensor.matmul(pso, kq0[:, 1, ch], S_sb[:, 0:256],
                                 start=True, stop=False)
                nc.tensor.matmul(pso, kq1[:, 1, ch], S_sb[:, 256:512],
                                 start=False, stop=False)
                nc.tensor.matmul(pso, st["Hm"], w, start=False, stop=True)
            else:
                nc.tensor.matmul(pso, st["Hm"], w, start=True, stop=True)
            st["pso"] = pso

        def emit_trail1(i):
            st = state[i]
            pso = st["pso"]
            o_sb = pcm.tile([128, 256], BF16, tag="o_sb", name="o_sb", bufs=8)
            with tc.high_priority():
                # frees the shared psW/pso bank the next chain step reuses
                _tag(nc.vector.tensor_copy(o_sb, pso), f"osb_{i}")
            st["o_sb"] = o_sb
            if dbg:
                nc.sync.dma_start(
                    out=dbgo_d.ap()[:, i * 256 : (i + 1) * 256], in_=o_sb
                )
            # rms sums via Act square-with-accumulate (set-18 resident)
            scratch = pcm.tile([128, 256], BF16, tag="scr2", name="scratch",
                               bufs=4)
            nc.scalar.activation(
                out=scratch, in_=o_sb, func=AF.Square,
                accum_out=sums_c[:, i : i + 1],
            )
            # m = sums/D + eps5*(sum_q + eps6)
            nc.vector.scalar_tensor_tensor(
                out=m_c[:, i : i + 1], in0=sums_c[:, i : i + 1],
                scalar=1.0 / D, in1=e5q_c[:, i : i + 1],
                op0=OP.mult, op1=OP.add,
            )

        def emit_rs(q):
            sl4 = slice(4 * q, 4 * q + 4)
            t1 = pcrow.tile([128, 4], F32, tag="t1", name="t1rs")
            t2 = pcrow.tile([128, 4], F32, tag="t2", name="t2rs")
            rsqrt_emit(rs_c[:, sl4], m_c[:, sl4], 1, t1[:, 0:4], t2[:, 0:4])

        def emit_trail2(i):
            st = state.pop(i)
            o_sb = st["o_sb"]
            tOT = ppt.tile([128, 1024], BF16, tag="ps", name="tOT")
            psOT = tOT[:, 0:256]
            nc.tensor.transpose(psOT[:, 0:128], o_sb[:, 0:128], identb)
            nc.tensor.transpose(psOT[:, 128:256], o_sb[:, 128:256], identb)
            ot = pcm.tile([128, 256], BF16, tag="ot", name="ot", bufs=4)
            nc.vector.tensor_copy(ot, psOT)

            outbuf = pout.tile([128, HID], F32, tag="outbuf", name="outbuf",
                               bufs=4)
            for hc in range(2):
                psop = ppt.tile([128, 512], F32, tag="ps", name="psop")
                nc.tensor.matmul(
                    psop, ot[:, 0:128], wo_sb[:, 0, hc * 512 : (hc + 1) * 512],
                    start=True, stop=False,
                )
                nc.tensor.matmul(
                    psop, ot[:, 128:256], wo_sb[:, 1, hc * 512 : (hc + 1) * 512],
                    start=False, stop=True,
                )
                dst = outbuf[:, hc * 512 : (hc + 1) * 512]
                if hc == 0:
                    nc.scalar.activation(out=dst, in_=psop, func=AF.Copy,
                                         scale=rs_c[:, i : i + 1])
                else:
                    nc.vector.tensor_scalar(
                        out=dst, in0=psop, scalar1=rs_c[:, i : i + 1],
                        scalar2=None, op0=OP.mult,
                    )
            ch = slice(i * C, (i + 1) * C)
            nc.sync.dma_start(out=out_d.ap()[ch, :], in_=outbuf)

        def emit_dbg():
            if not dbg:
                return
            for j, tile_ in enumerate((beta_c, ak_c, nbk2_c, e5q_c, m_c,
                                       rs_c)):
                nc.sync.dma_start(
                    out=dbgsc_d.ap()[:, j * nchunk : (j + 1) * nchunk],
                    in_=tile_,
                )

        nquad = nchunk // 4
        LAQ = max(1, int(os.environ.get("KLOOKAHEAD", 2)))

        def emit_stage(q):
            emit_slab_B(q)
            emit_scalars(q)
            emit_pass1_quad(q)

        for q in range(min(LAQ, nquad)):
            emit_stage(q)
        for i in range(nchunk):
            emit_chain(i)
            emit_trail1(i)
            if i % 4 == 3:
                q = i // 4
                if q + LAQ < nquad:
                    emit_stage(q + LAQ)
                emit_rs(q)
                for j in range(i - 3, i + 1):
                    emit_trail2(j)
        emit_dbg()

    nc.compile()
    return nc


def make_host_inputs(inputs, nchunk=S_FULL // C):
    S = nchunk * C
    bf = ml_dtypes.bfloat16
    hs = np.asarray(inputs["hidden_states"], np.float32)[:, :S, :]
    Wq, Wk, Wv = (np.asarray(inputs[k], np.float32) for k in ("Wq", "Wk", "Wv"))
    Wb = np.asarray(inputs["Wb"], np.float32)
    Wo = np.asarray(inputs["Wo"], np.float32)
    nw = np.asarray(inputs["norm_w"], np.float32)
    convs = {
        k: np.asarray(inputs[k], np.float32) for k in ("conv_q", "conv_k", "conv_v")
    }

    identb = np.eye(128, dtype=np.float32)
    onescol = np.ones((128, 1), np.float32)
    mlow = np.tril(np.ones((128, 128), np.float32), -1)
    mup = np.triu(np.ones((128, 128), np.float32), 0)

    def diag_pack(cw):
        out = np.zeros((128, 8 * 128), np.float32)
        for j in range(4):
            for dt_ in range(2):
                blk = np.diag(cw[dt_ * 128 : (dt_ + 1) * 128, j])
                out[:, (j * 2 + dt_) * 128 : (j * 2 + dt_ + 1) * 128] = blk
        return out

    def c(a, dt=bf):
        return np.ascontiguousarray(a).astype(dt)

    in_maps = []
    for core in range(8):
        b, h = core // 4, core % 4
        hsel = slice(h * D, (h + 1) * D)
        in_maps.append(
            {
                "xt": c(hs[b].T),
                "wq": c(Wq[:, hsel]),
                "wk": c(Wk[:, hsel]),
                "wv": c(Wv[:, hsel]),
                "wb": c(Wb[:, h : h + 1]),
                "wo": c(nw[:, None] * Wo[hsel, :]),
                "cdq": c(diag_pack(convs["conv_q"][hsel])),
                "cdk": c(diag_pack(convs["conv_k"][hsel])),
                "cdv": c(diag_pack(convs["conv_v"][hsel])),
                "identb": c(identb),
                "identb2": c(np.concatenate([identb, identb], axis=1)),
                "identb4": c(np.concatenate([identb] * 4, axis=1)),
                "onescol": c(onescol),
                "mlow": c(mlow, np.float32),
                "mup": c(mup, np.float32),
            }
        )
    return in_maps


_NC_CACHE = {}


def _get_nc(nchunk):
    if nchunk not in _NC_CACHE:
        _NC_CACHE[nchunk] = build_nc(nchunk)
    return _NC_CACHE[nchunk]


def kernel(**inputs) -> np.ndarray:
    nchunk = S_FULL // C
    nc = _get_nc(nchunk)
    in_maps = make_host_inputs(inputs, nchunk)
    res = run_bass_kernel_spmd(nc, in_maps, core_ids=list(range(8)))
    S = nchunk * C
    out = np.zeros((B, S, HID), np.float32)
    for core in range(8):
        out[core // 4] += np.asarray(res.results[core]["out"], np.float32)
    return out
